# revision 63
# baseline (speedup 1.0000x reference)
"""Trainium2 Bass kernel for nn_LASLNNet (complex-valued 4D CNN).

Strategy (8 NeuronCores, SPMD single program):
  - core c handles (batch b = c//2, spatial half h = c%2) -> 4 x 2 split.
  - All complex convs are computed as real matmuls with doubled channels:
      [yr; yi] = [[Wr, Wi], [-Wi, Wr]]^T @ [xr; xi]
  - conv1 (k=3,s=2): im2col-lite slabs prepared on host (27 (j1,j2,j3) tap
    slabs; j4 handled as 3 PSUM-accumulated matmuls with step-2 rhs reads).
    Bias folded in via an all-ones K-channel so dummy edge rows stay zero.
  - conv2 (k=3,s=1,p=1): input stored on a d4-padded flat grid
    [block(d1) 7, d2 9, d3 9, d4 10] so each (j1,j2,j3) tap is a single
    flat offset; j4 in {0,1} fused into one K=128 matmul via a 1-element
    shifted replica of the input on partitions 64..127; j4=2 is a K=128
    matmul at base+2 whose weight rows 64..127 are zero (keeping K=128
    keeps FWL fast-weight-load active; true K=64 matmuls are ~3x slower
    on the weight port), with o4 clipped to [0,8) since o4=8 only reads
    the zero pad. Edge taps restrict (o2,o3) ranges via strided APs; PSUM
    has_written semantics make partial-region accumulation correct
    (the first matmul of each group is the full-region interior tap).
    Matmuls run tap-major over row-pairs so one weight load serves 4
    matmuls (LDWEIGHTS 540->162).
  - conv3/4/5 (1x1): plain matmuls, chunk-major so each stage's chunk c
    unblocks the next stage early; relu+bias drains alternate
    ScalarE/VectorE so neither engine is the stage bottleneck.
  - FC: on-chip mul+reduce against per-core-masked fcw; final cross-half
    sum + fc bias on host (each core returns a [128,1] partial).
  - dtype: bf16 matmul operands, fp32 PSUM/copies.
  - DMA discipline: each dma_start costs ~1-2.5us of HWDGE ring occupancy
    beyond the transfer, so loads are consolidated (small tensors packed
    into packf/packb) and split across both HWDGE rings (SP: w1+x1,
    ACT: w2+packs) in first-use order; the conv1->conv2 shifted-replica
    SBUF copy is 2 chunked DMAs so conv2 row 0 starts early.

Spatial split along first output spatial dim D1 (9 rows):
  half 0 -> conv2..4 rows 0..4, half 1 -> rows 4..8 (row 4 duplicated);
  conv5 rows {0,1,2} / {2,3,4} (row 2 duplicated, masked via zeroed fcw).

Measured per-execution HW time (slope of a For_i-wrapped build between
loop counts 8 and 264, interleaved sampling): ~126us, conv2-dominated
(~70-80us of matmul streaming near its N/2.4GHz stream-limit).
"""

import itertools

import numpy as np
import ml_dtypes

import concourse.bacc as bacc
import concourse.mybir as mybir
from concourse.tile import TileContext
from concourse.bass_utils import run_bass_kernel_spmd

F32 = mybir.dt.float32
BF16 = mybir.dt.bfloat16
BF = ml_dtypes.bfloat16

NB = 4            # batch
R1 = 7            # conv1 rows computed per core (incl. dummy edge rows)
R2 = 5            # conv2/3/4 rows per core
R5 = 3            # conv5 rows per core
D4P = 10          # d4-padded inner dim (9 valid + 1 zero)
BLK = 9 * 9 * D4P                # 810, one d1-block of x2
X2N = R1 * BLK                   # logical x2 elements per partition
S1N = R1 * 9 * 9 * 20            # 11340 conv1 slab elements per partition
N3 = R2 * 729                    # 3645 compact columns for conv3/4
N5 = R5 * 125                    # 375 conv5 output columns

_CACHE = {}


def _build_nc(loop_n=1, _stage="full"):
    """Build the kernel. loop_n>1 wraps the whole per-execution body
    (input DMA loads, compute, output store) in a hardware For_i loop so
    steady-state per-execution HW time can be measured as a slope.
    _stage truncates the body after a pipeline stage (ablation only)."""
    import contextlib
    nc = bacc.Bacc("TRN2", target_bir_lowering=False, debug=False)

    # DMA count is minimized: each dma_start costs ~1-2.5us of HWDGE ring
    # occupancy beyond the raw transfer, so small tensors are packed into
    # two pack tensors (packf: f32 biases+fcw; packb: bf16 w3/w4/w5).
    x1_d = nc.dram_tensor("x1", [64, S1N], BF16, kind="ExternalInput")
    w1_d = nc.dram_tensor("w1", [64, 3 * 64], BF16, kind="ExternalInput")
    w2a_d = nc.dram_tensor("w2a", [128, 27 * 128], BF16, kind="ExternalInput")
    # w2b zero-padded to K=128 (rows 64..127 = 0 from host): K=64 matmuls
    # disable FWL (fast weight load) and run ~4x slower on the weight port.
    w2b_d = nc.dram_tensor("w2b", [128, 27 * 128], BF16, kind="ExternalInput")
    # packf cols: [0]=b2, [1:3]=b3, [3:5]=b4, [5:6]=b5, [6:381]=fcw
    packf_d = nc.dram_tensor("packf", [128, 6 + N5], F32,
                             kind="ExternalInput")
    # packb cols: [0:256]=w3, [256:768]=w4, [768:1024]=w5
    packb_d = nc.dram_tensor("packb", [128, 1024], BF16,
                             kind="ExternalInput")
    out_d = nc.dram_tensor("out", [128, 1], F32, kind="ExternalOutput")

    Relu = mybir.ActivationFunctionType.Relu

    with TileContext(nc) as tc:
        with tc.tile_pool(name="sb", bufs=1) as pool, \
             tc.tile_pool(name="ps", bufs=8, space="PSUM") as pp:
            x1t = pool.tile([64, S1N], BF16, tag="x1")
            w1t = pool.tile([64, 3 * 64], BF16, tag="w1")
            # x2 store: [1 lead margin][R1 blocks of BLK][1 tail margin]
            x2t = pool.tile([128, X2N + 92], BF16, tag="x2")
            w2at = pool.tile([128, 27 * 128], BF16, tag="w2a")
            w2bt = pool.tile([128, 27 * 128], BF16, tag="w2b")
            packft = pool.tile([128, 6 + N5], F32, tag="packf")
            packbt = pool.tile([128, 1024], BF16, tag="packb")
            x3t = pool.tile([128, N3], BF16, tag="x3")
            x4t = pool.tile([128, 2 * N3], BF16, tag="x4")
            x4bt = pool.tile([128, 2 * N3], BF16, tag="x4b")
            x5t = pool.tile([128, N5], F32, tag="x5")
            prodt = pool.tile([128, N5], F32, tag="prod")
            fct = pool.tile([128, 1], F32, tag="fc")
            wut = pool.tile([128, 512], BF16, tag="wu")

            loop_cm = (tc.For_i(0, loop_n) if loop_n > 1
                       else contextlib.nullcontext())
            with loop_cm:
                _build_body(nc, tc, pp, locals(), _stage)

    nc.compile()
    return nc


def _build_body(nc, tc, pp, lv, stage="full"):
    Relu = mybir.ActivationFunctionType.Relu
    (x1t, w1t, x2t, w2at, w2bt, packft, packbt, x3t, x4t, x4bt,
     x5t, prodt, fct, wut) = (
        lv["x1t"], lv["w1t"], lv["x2t"], lv["w2at"], lv["w2bt"],
        lv["packft"], lv["packbt"], lv["x3t"], lv["x4t"], lv["x4bt"],
        lv["x5t"], lv["prodt"], lv["fct"], lv["wut"])
    (x1_d, w1_d, w2a_d, w2b_d, packf_d, packb_d, out_d) = (
        lv["x1_d"], lv["w1_d"], lv["w2a_d"], lv["w2b_d"], lv["packf_d"],
        lv["packb_d"], lv["out_d"])
    if True:
            # PE warm-up burst: the PE clock-gate (HAM) starts at 1.2 GHz and
            # only reaches 2.4 GHz after ~3.4us of sustained activity. The
            # head DMA phase would leave the PE idle anyway, so stream dummy
            # matmuls on a scratch tile to trip the gate before conv1.
            nc.vector.memset(wut[:, :], 0)
            psw = pp.tile([128, 512], F32, tag="ps")
            for wi in range(10):
                nc.tensor.matmul(psw[:, :], wut[:, 0:128], wut[:, :],
                                 start=True, stop=True)
            # SP HWDGE ring: w1 first (conv1's first dependency, tiny), then
            # x1 chunked so early conv1 rows can start before the whole slab
            # lands (Tile subtile deps); first chunk covers rows 0-2, which
            # unblock conv2 row 0.
            nc.sync.dma_start(w1t[:, :], w1_d[:, :])
            for (rlo, rhi) in ((0, 3), (3, 5), (5, 7)):
                nc.sync.dma_start(x1t[:, rlo * 1620:rhi * 1620],
                                  x1_d[:, rlo * 1620:rhi * 1620])
            # ACT HWDGE ring (parallel with SP): w2 in natural tap order
            # (conv2 consumes taps 4,0,1,2,...), interleaved a/b in thirds
            # so every slice lands ~3-7us before conv2 consumes it, then the
            # packed small tensors.
            for (tl, th) in ((0, 9), (9, 18), (18, 27)):
                nc.scalar.dma_start(w2at[:, tl * 128:th * 128],
                                    w2a_d[:, tl * 128:th * 128])
                nc.scalar.dma_start(w2bt[:, tl * 128:th * 128],
                                    w2b_d[:, tl * 128:th * 128])
            nc.scalar.dma_start(packft[:, :], packf_d[:, :])
            nc.scalar.dma_start(packbt[:, :], packb_d[:, :])

            # zero x2: lead margin, tail margin, d4 pad columns. The valid
            # [blk, d2, d3, 0:9] region is fully written by conv1 (dummy edge
            # rows produce exact zeros via the all-ones bias channel).
            nc.vector.memset(x2t[:, 0:1], 0)
            nc.vector.memset(x2t[:, 1 + X2N:X2N + 92], 0)
            x2pad = x2t[:, 1:1 + X2N].rearrange(
                "p (r c) -> p r c", r=R1 * 81, c=D4P)[:, :, 9:10]
            nc.vector.memset(x2pad, 0)

            if stage == "dma":
                nc.vector.memset(fct[:, :], 0)
                nc.sync.dma_start(out_d[:, :], fct[:, :])
                return

            # ---------------- conv1 ----------------
            # slab view: [r(R1), o2(9), o3(9), d4(20)]
            s1v = x1t.rearrange("p (r a b c) -> p r a b c", r=R1, a=9, b=9, c=20)
            # x2 logical view (alloc offset 1): [blk(R1), d2(9), d3(9), d4(D4P)]
            x2v = x2t[:, 1:1 + X2N].rearrange(
                "p (r a b c) -> p r a b c", r=R1, a=9, b=9, c=D4P)
            for r in range(R1):
                for gi, (o2s, c2g) in enumerate(((0, 5), (5, 4))):
                    n = c2g * 81
                    ps1 = pp.tile([128, 512], F32, tag="ps")
                    ps1v = ps1[0:64, :n].rearrange("p (a b c) -> p a b c",
                                                   a=c2g, b=9, c=9)
                    for j4 in range(3):
                        rhs = s1v[:, r, o2s:o2s + c2g, :, j4:j4 + 17:2]
                        nc.tensor.matmul(
                            ps1v[:, :, :, :],
                            w1t[:, j4 * 64:(j4 + 1) * 64],
                            rhs,
                            start=(j4 == 0), stop=(j4 == 2))
                    # (NOTE: splitting these drains ScalarE/VectorE measured
                    # +12us — DVE with a strided PSUM source is much slower
                    # than ACT here; keep all conv1 drains on ScalarE.)
                    nc.scalar.activation(
                        x2v[0:64, r, o2s:o2s + c2g, :, 0:9],
                        ps1v[:, :, :, :],
                        Relu)
                # shifted replica for conv2 j4-fusion, in two chunks so conv2
                # row 0 (blocks 0-2) can start early while conv1 rows 3-6
                # still run: x2t[64+p, a] = x2t[p, a+1]. Chunks are
                # self-contained; the col a=X2N boundary is pad (zero).
                if r in (2, R1 - 1):
                    clo = 0 if r == 2 else 3 * BLK
                    chi = 3 * BLK if r == 2 else R1 * BLK
                    nc.sync.dma_start(x2t[64:128, clo:chi],
                                      x2t[0:64, 1 + clo:1 + chi])

            if stage == "c1":
                nc.vector.memset(fct[:, :], 0)
                nc.sync.dma_start(out_d[:, :], fct[:, :])
                return

            # ---------------- conv2 ----------------
            # taps ordered interior-first so the first matmul of each PSUM
            # group covers the full region (has_written correctness).
            # (0,1,1) is interior in (j2,j3) — full region — and its t27=4
            # sits in the first w2 DMA chunk, so the natural-order stream
            # pipelines behind the weight loads.
            taps = sorted(itertools.product(range(3), repeat=3),
                          key=lambda t: (t != (0, 1, 1)))
            x3v = x3t.rearrange("p (r a b c) -> p r a b c", r=R2, a=9, b=9, c=9)
            G2 = ((0, 5), (5, 4))

            def c2geom(j1, j2, j3, r, o2s, c2g):
                lo2 = max(o2s, 1 - j2)
                hi2 = min(o2s + c2g, 10 - j2)
                lo3 = max(0, 1 - j3)
                hi3 = min(9, 10 - j3)
                c2, c3 = hi2 - lo2, hi3 - lo3
                # alloc base for (o2=lo2, o3=lo3, o4=0), j4=0 on the base
                # partitions (the +1 alloc offset and the -1 d4 pad shift
                # cancel):
                base0 = ((r + j1) * BLK + (lo2 + j2 - 1) * 90
                         + (lo3 + j3 - 1) * D4P)
                return lo2, hi2, lo3, hi3, c2, c3, base0

            # Tap-major over row-pairs: one weight load serves 4 matmuls
            # (2 rows x 2 column groups), cutting LDWEIGHTS count 540->162
            # and per-matmul DMA-semaphore waits 4x.
            for rset in ((0, 1), (2, 3), (4,)):
                pss = {}
                for r in rset:
                    for gi, (o2s, c2g) in enumerate(G2):
                        ps2 = pp.tile([128, 512], F32, tag="ps")
                        pss[(r, gi)] = ps2[:, :c2g * 81].rearrange(
                            "p (a b c) -> p a b c", a=c2g, b=9, c=9)
                for ti, (j1, j2, j3) in enumerate(taps):
                    t27 = j1 * 9 + j2 * 3 + j3
                    wa = w2at[:, t27 * 128:(t27 + 1) * 128]
                    wb = w2bt[:, t27 * 128:(t27 + 1) * 128]
                    # ti==0: pair matmul first (full region carries start /
                    # has_written); ti==26: pair matmul last (carries stop).
                    # j4=2 matmul: K=128 at base+2 (w2b rows 64..127 are
                    # zero so the shifted-replica partitions contribute 0),
                    # o4 clipped to [0,8) since o4=8 only reads the d4 pad.
                    for wsel in ((0, 1) if ti == 0 else (1, 0)):
                        for r in rset:
                            for gi, (o2s, c2g) in enumerate(G2):
                                (lo2, hi2, lo3, hi3, c2, c3,
                                 base0) = c2geom(j1, j2, j3, r, o2s, c2g)
                                if wsel == 0:
                                    rhs = x2t[:, base0:base0 +
                                              c2 * 90].rearrange(
                                        "p (a b c) -> p a b c",
                                        a=c2, b=9, c=D4P)[:, :, 0:c3, 0:9]
                                    nc.tensor.matmul(
                                        pss[(r, gi)][:, lo2 - o2s:hi2 - o2s,
                                                     lo3:hi3, :],
                                        wa, rhs, start=(ti == 0),
                                        stop=(ti == 26))
                                else:
                                    rhs = x2t[:, base0 + 2:base0 + 2 +
                                              c2 * 90].rearrange(
                                        "p (a b c) -> p a b c",
                                        a=c2, b=9, c=D4P)[:, :, 0:c3, 0:8]
                                    nc.tensor.matmul(
                                        pss[(r, gi)][:, lo2 - o2s:hi2 - o2s,
                                                     lo3:hi3, 0:8],
                                        wb, rhs, start=False, stop=False)
                for r in rset:
                    for gi, (o2s, c2g) in enumerate(G2):
                        nc.scalar.activation(
                            x3v[:, r, o2s:o2s + c2g, :, :],
                            pss[(r, gi)][:, :, :, :],
                            Relu, bias=packft[:, 0:1])

            if stage == "c2":
                nc.vector.memset(fct[:, :], 0)
                nc.sync.dma_start(out_d[:, :], fct[:, :])
                return

            # ---------------- conv3 (1x1, 64c->128c) ----------------
            chunks = []
            pos = 0
            while pos < N3:
                sz = min(512, N3 - pos)
                chunks.append((pos, sz))
                pos += sz
            # chunk-major (mh inner) so conv4's chunk c — which needs BOTH mh
            # halves of x4t chunk c — can start right after conv3 chunk c.
            # Drains alternate ScalarE/VectorE to split the PSUM->SBUF
            # relu+bias work across both engines.
            Add, Max = mybir.AluOpType.add, mybir.AluOpType.max
            for ci, (pos, sz) in enumerate(chunks):
                for mh in range(2):
                    ps3 = pp.tile([128, 512], F32, tag="ps")
                    nc.tensor.matmul(
                        ps3[:, :sz],
                        packbt[:, mh * 128:(mh + 1) * 128],
                        x3t[:, pos:pos + sz],
                        start=True, stop=True)
                    dst = x4t[:, mh * N3 + pos:mh * N3 + pos + sz]
                    if mh == 0:
                        nc.scalar.activation(dst, ps3[:, :sz],
                                             Relu, bias=packft[:, 1 + mh:2 + mh])
                    else:
                        nc.vector.tensor_scalar(dst, ps3[:, :sz],
                                                packft[:, 1 + mh:2 + mh], 0.0,
                                                Add, Max)

            if stage == "c3":
                nc.vector.memset(fct[:, :], 0)
                nc.sync.dma_start(out_d[:, :], fct[:, :])
                return

            # ---------------- conv4 (1x1, 128c->128c) ----------------
            for ci, (pos, sz) in enumerate(chunks):
                for mh in range(2):
                    ps4 = pp.tile([128, 512], F32, tag="ps")
                    nc.tensor.matmul(
                        ps4[:, :sz],
                        packbt[:, 256 + (mh * 2) * 128:256 + (mh * 2 + 1) * 128],
                        x4t[:, pos:pos + sz],
                        start=True, stop=False)
                    nc.tensor.matmul(
                        ps4[:, :sz],
                        packbt[:, 256 + (mh * 2 + 1) * 128:256 + (mh * 2 + 2) * 128],
                        x4t[:, N3 + pos:N3 + pos + sz],
                        start=False, stop=True)
                    dst = x4bt[:, mh * N3 + pos:mh * N3 + pos + sz]
                    if mh == 0:
                        nc.scalar.activation(dst, ps4[:, :sz],
                                             Relu, bias=packft[:, 3 + mh:4 + mh])
                    else:
                        nc.vector.tensor_scalar(dst, ps4[:, :sz],
                                                packft[:, 3 + mh:4 + mh], 0.0,
                                                Add, Max)

            if stage == "c4":
                nc.vector.memset(fct[:, :], 0)
                nc.sync.dma_start(out_d[:, :], fct[:, :])
                return

            # ---------------- conv5 (1x1, s=2, 128c->64c) ----------------
            # x4b view: [mb(2), r(R2), o2(9), o3(9), o4(9)]
            x4bv = x4bt.rearrange("p (m r a b c) -> p m r a b c",
                                  m=2, r=R2, a=9, b=9, c=9)
            for rr in range(R5):
                ps5 = pp.tile([128, 512], F32, tag="ps")
                for mb in range(2):
                    rhs = x4bv[:, mb, 2 * rr, 0:9:2, 0:9:2, 0:9:2]
                    nc.tensor.matmul(
                        ps5[:, :125],
                        packbt[:, 768 + mb * 128:768 + (mb + 1) * 128],
                        rhs,
                        start=(mb == 0), stop=(mb == 1))
                nc.scalar.activation(
                    x5t[:, rr * 125:(rr + 1) * 125],
                    ps5[:, :125],
                    Relu, bias=packft[:, 5:6])

            if stage == "c5":
                nc.vector.memset(fct[:, :], 0)
                nc.sync.dma_start(out_d[:, :], fct[:, :])
                return

            # ---------------- FC partials ----------------
            # (tensor_tensor_reduce would fuse these but faults in this
            # execution path)
            nc.vector.tensor_mul(prodt[:, :], x5t[:, :], packft[:, 6:6 + N5])
            nc.vector.reduce_sum(fct[:, :], prodt[:, :],
                                 axis=mybir.AxisListType.X)

            nc.sync.dma_start(out_d[:, :], fct[:, :], single_packet=True)


# ---------------- host-side data prep ----------------

def _prep_weights(inputs):
    f32 = np.float32
    w1r = np.asarray(inputs["w1r"], f32)[:, 0]   # [32, 3,3,3,3]
    w1i = np.asarray(inputs["w1i"], f32)[:, 0]
    # [t27, j4, co]
    w1r_t = w1r.transpose(1, 2, 3, 4, 0).reshape(27, 3, 32)
    w1i_t = w1i.transpose(1, 2, 3, 4, 0).reshape(27, 3, 32)
    W1 = np.zeros((64, 3 * 64), f32)
    for j4 in range(3):
        W1[0:27, j4 * 64:j4 * 64 + 32] = w1r_t[:, j4]
        W1[0:27, j4 * 64 + 32:j4 * 64 + 64] = w1i_t[:, j4]
        W1[27:54, j4 * 64:j4 * 64 + 32] = -w1i_t[:, j4]
        W1[27:54, j4 * 64 + 32:j4 * 64 + 64] = w1r_t[:, j4]
    W1[54, 0:32] = np.asarray(inputs["b1r"], f32)
    W1[54, 32:64] = np.asarray(inputs["b1i"], f32)

    w2r = np.asarray(inputs["w2r"], f32)   # [64, 32, 3,3,3,3]
    w2i = np.asarray(inputs["w2i"], f32)
    # [t27, j4, ci, co]
    w2r_t = w2r.transpose(2, 3, 4, 5, 1, 0).reshape(27, 3, 32, 64)
    w2i_t = w2i.transpose(2, 3, 4, 5, 1, 0).reshape(27, 3, 32, 64)
    W2a = np.zeros((128, 27 * 128), f32)
    W2b = np.zeros((128, 27 * 128), f32)  # rows 64..127 stay 0 (K=128 pad)
    for t in range(27):
        for jj, r0 in ((0, 0), (1, 64)):
            W2a[r0 + 0:r0 + 32, t * 128:t * 128 + 64] = w2r_t[t, jj]
            W2a[r0 + 0:r0 + 32, t * 128 + 64:(t + 1) * 128] = w2i_t[t, jj]
            W2a[r0 + 32:r0 + 64, t * 128:t * 128 + 64] = -w2i_t[t, jj]
            W2a[r0 + 32:r0 + 64, t * 128 + 64:(t + 1) * 128] = w2r_t[t, jj]
        W2b[0:32, t * 128:t * 128 + 64] = w2r_t[t, 2]
        W2b[0:32, t * 128 + 64:(t + 1) * 128] = w2i_t[t, 2]
        W2b[32:64, t * 128:t * 128 + 64] = -w2i_t[t, 2]
        W2b[32:64, t * 128 + 64:(t + 1) * 128] = w2r_t[t, 2]
    B2 = np.concatenate([np.asarray(inputs["b2r"], f32),
                         np.asarray(inputs["b2i"], f32)])[:, None]

    w3r = np.asarray(inputs["w3r"], f32).reshape(128, 64)
    w3i = np.asarray(inputs["w3i"], f32).reshape(128, 64)
    W3 = np.zeros((128, 2 * 128), f32)
    W3[0:64, 0:128] = w3r.T
    W3[64:128, 0:128] = -w3i.T
    W3[0:64, 128:256] = w3i.T
    W3[64:128, 128:256] = w3r.T
    B3 = np.stack([np.asarray(inputs["b3r"], f32),
                   np.asarray(inputs["b3i"], f32)], axis=1)

    w4r = np.asarray(inputs["w4r"], f32).reshape(128, 128)
    w4i = np.asarray(inputs["w4i"], f32).reshape(128, 128)
    W4 = np.zeros((128, 4 * 128), f32)
    W4[:, 0:128] = w4r.T
    W4[:, 128:256] = -w4i.T
    W4[:, 256:384] = w4i.T
    W4[:, 384:512] = w4r.T
    B4 = np.stack([np.asarray(inputs["b4r"], f32),
                   np.asarray(inputs["b4i"], f32)], axis=1)

    w5r = np.asarray(inputs["w5r"], f32).reshape(64, 128)
    w5i = np.asarray(inputs["w5i"], f32).reshape(64, 128)
    W5 = np.zeros((128, 2 * 128), f32)
    W5[:, 0:64] = w5r.T
    W5[:, 64:128] = w5i.T
    W5[:, 128:192] = -w5i.T
    W5[:, 192:256] = w5r.T
    B5 = np.concatenate([np.asarray(inputs["b5r"], f32),
                         np.asarray(inputs["b5i"], f32)])[:, None]

    # packf cols: [0]=b2, [1:3]=b3, [3:5]=b4, [5:6]=b5 (fcw appended
    # per-core by _mk_packf); packb cols: [0:256]=w3, [256:768]=w4,
    # [768:1024]=w5
    packf0 = np.concatenate([B2, B3, B4, B5], axis=1).astype(f32)
    packb = np.concatenate([W3, W4, W5], axis=1).astype(BF)
    return {
        "w1": W1.astype(BF), "w2a": W2a.astype(BF), "w2b": W2b.astype(BF),
        "packf0": packf0, "packb": packb,
    }


def _mk_packf(packf0, fcw, h):
    return np.concatenate([packf0, _prep_fcw(fcw, h)], axis=1)


def _prep_x1(xr_b, xi_b, h):
    """Conv1 input slab for one (batch, half): [64, R1, 9, 9, 20] bf16."""
    S = np.zeros((64, R1, 9, 9, 20), np.float32)
    glo = max(0, 4 * h - 1)
    ghi = min(8, 4 * h + 5)
    rlo = glo - (4 * h - 1)
    rhi = ghi - (4 * h - 1) + 1
    for t, (j1, j2, j3) in enumerate(itertools.product(range(3), repeat=3)):
        subr = xr_b[j1:j1 + 17:2, j2:j2 + 17:2, j3:j3 + 17:2, :]
        subi = xi_b[j1:j1 + 17:2, j2:j2 + 17:2, j3:j3 + 17:2, :]
        S[t, rlo:rhi] = subr[glo:ghi + 1]
        S[27 + t, rlo:rhi] = subi[glo:ghi + 1]
    S[54, rlo:rhi] = 1.0
    return S.reshape(64, S1N).astype(BF)


def _prep_fcw(fcw, h):
    out = np.zeros((128, N5), np.float32)
    f = np.asarray(fcw, np.float32).reshape(-1)
    for rr in range(R5):
        g5 = rr + 2 * h
        if h == 1 and rr == 0:
            continue  # overlap row masked on half 1
        out[:, rr * 125:(rr + 1) * 125] = f[g5 * 125:(g5 + 1) * 125][None, :]
    return out


def kernel(**inputs):
    if "nc" not in _CACHE:
        _CACHE["nc"] = _build_nc()
    nc = _CACHE["nc"]

    wmaps = _prep_weights(inputs)
    xr = np.asarray(inputs["xr"], np.float32)
    xi = np.asarray(inputs["xi"], np.float32)
    fcw = inputs["fcw"]

    in_maps = []
    for core in range(8):
        b, h = core // 2, core % 2
        m = {"w1": wmaps["w1"], "w2a": wmaps["w2a"], "w2b": wmaps["w2b"],
             "packb": wmaps["packb"]}
        m["x1"] = _prep_x1(xr[b, 0], xi[b, 0], h)
        m["packf"] = _mk_packf(wmaps["packf0"], fcw, h)
        in_maps.append(m)

    res = run_bass_kernel_spmd(nc, in_maps, core_ids=list(range(8)))

    fcb = np.asarray(inputs["fcb"], np.float32)
    yr = np.zeros((NB, 64, 1), np.float32)
    yi = np.zeros((NB, 64, 1), np.float32)
    for b in range(NB):
        p0 = res.results[2 * b]["out"]
        p1 = res.results[2 * b + 1]["out"]
        s = p0 + p1
        yr[b] = s[0:64] + fcb[0]
        yi[b] = s[64:128]
    return np.stack([yr, yi]).astype(np.float32)



# revision 64
# speedup vs baseline: 1.0212x; 1.0212x over previous
"""Trainium2 Bass kernel for nn_LASLNNet (complex-valued 4D CNN).

Strategy (8 NeuronCores, SPMD single program):
  - core c handles (batch b = c//2, spatial half h = c%2) -> 4 x 2 split.
  - All complex convs are computed as real matmuls with doubled channels:
      [yr; yi] = [[Wr, Wi], [-Wi, Wr]]^T @ [xr; xi]
  - conv1 (k=3,s=2): im2col-lite slabs prepared on host (27 (j1,j2,j3) tap
    slabs; j4 handled as 3 PSUM-accumulated matmuls with step-2 rhs reads).
    Bias folded in via an all-ones K-channel so dummy edge rows stay zero.
  - conv2 (k=3,s=1,p=1): input stored on a d4-padded flat grid
    [block(d1) 7, d2 9, d3 9, d4 10] so each (j1,j2,j3) tap is a single
    flat offset; j4 in {0,1} fused into one K=128 matmul via a 1-element
    shifted replica of the input on partitions 64..127; j4=2 is a K=128
    matmul at base+2 whose weight rows 64..127 are zero (keeping K=128
    keeps FWL fast-weight-load active; true K=64 matmuls are ~3x slower
    on the weight port), with o4 clipped to [0,8) since o4=8 only reads
    the zero pad. Edge taps restrict (o2,o3) ranges via strided APs; PSUM
    has_written semantics make partial-region accumulation correct
    (the first matmul of each group is the full-region interior tap).
    Matmuls run tap-major over row-pairs so one weight load serves 4
    matmuls (LDWEIGHTS 540->162).
  - conv3/4/5 (1x1): plain matmuls, chunk-major so each stage's chunk c
    unblocks the next stage early; relu+bias drains alternate
    ScalarE/VectorE so neither engine is the stage bottleneck.
  - FC: on-chip mul+reduce against per-core-masked fcw; final cross-half
    sum + fc bias on host (each core returns a [128,1] partial).
  - dtype: bf16 matmul operands, fp32 PSUM/copies.
  - DMA discipline: each dma_start costs ~1-2.5us of HWDGE ring occupancy
    beyond the transfer, so loads are consolidated (small tensors packed
    into packf/packb) and split across both HWDGE rings (SP: w1+x1,
    ACT: w2+packs) in first-use order; the conv1->conv2 shifted-replica
    SBUF copy is 2 chunked DMAs so conv2 row 0 starts early.

Spatial split along first output spatial dim D1 (9 rows):
  half 0 -> conv2..4 rows 0..4, half 1 -> rows 4..8 (row 4 duplicated);
  conv5 rows {0,1,2} / {2,3,4} (row 2 duplicated, masked via zeroed fcw).

Measured per-execution HW time (slope of a For_i-wrapped build between
loop counts 8 and 264, interleaved sampling): ~126us, conv2-dominated
(~70-80us of matmul streaming near its N/2.4GHz stream-limit).
"""

import itertools

import numpy as np
import ml_dtypes

import concourse.bacc as bacc
import concourse.mybir as mybir
from concourse.tile import TileContext
from concourse.bass_utils import run_bass_kernel_spmd

F32 = mybir.dt.float32
BF16 = mybir.dt.bfloat16
BF = ml_dtypes.bfloat16

NB = 4            # batch
R1 = 7            # conv1 rows computed per core (incl. dummy edge rows)
R2 = 5            # conv2/3/4 rows per core
R5 = 3            # conv5 rows per core
D4P = 10          # d4-padded inner dim (9 valid + 1 zero)
BLK = 9 * 9 * D4P                # 810, one d1-block of x2
X2N = R1 * BLK                   # logical x2 elements per partition
S1N = R1 * 9 * 9 * 20            # 11340 conv1 slab elements per partition
N3 = R2 * 729                    # 3645 compact columns for conv3/4
N5 = R5 * 125                    # 375 conv5 output columns

_CACHE = {}


def _build_nc(loop_n=1, _stage="full"):
    """Build the kernel. loop_n>1 wraps the whole per-execution body
    (input DMA loads, compute, output store) in a hardware For_i loop so
    steady-state per-execution HW time can be measured as a slope.
    _stage truncates the body after a pipeline stage (ablation only)."""
    import contextlib
    nc = bacc.Bacc("TRN2", target_bir_lowering=False, debug=False)

    # DMA count is minimized: each dma_start costs ~1-2.5us of HWDGE ring
    # occupancy beyond the raw transfer, so small tensors are packed into
    # two pack tensors (packf: f32 biases+fcw; packb: bf16 w3/w4/w5).
    x1_d = nc.dram_tensor("x1", [64, S1N], BF16, kind="ExternalInput")
    w1_d = nc.dram_tensor("w1", [64, 3 * 64], BF16, kind="ExternalInput")
    w2a_d = nc.dram_tensor("w2a", [128, 27 * 128], BF16, kind="ExternalInput")
    # w2b zero-padded to K=128 (rows 64..127 = 0 from host): K=64 matmuls
    # disable FWL (fast weight load) and run ~4x slower on the weight port.
    w2b_d = nc.dram_tensor("w2b", [128, 27 * 128], BF16, kind="ExternalInput")
    # packf cols: [0]=b2, [1:3]=b3, [3:5]=b4, [5:6]=b5, [6:381]=fcw
    packf_d = nc.dram_tensor("packf", [128, 6 + N5], F32,
                             kind="ExternalInput")
    # packb cols: [0:256]=w3, [256:768]=w4, [768:1024]=w5
    packb_d = nc.dram_tensor("packb", [128, 1024], BF16,
                             kind="ExternalInput")
    out_d = nc.dram_tensor("out", [128, 1], F32, kind="ExternalOutput")

    Relu = mybir.ActivationFunctionType.Relu

    with TileContext(nc) as tc:
        with tc.tile_pool(name="sb", bufs=1) as pool, \
             tc.tile_pool(name="ps", bufs=8, space="PSUM") as pp:
            x1t = pool.tile([64, S1N], BF16, tag="x1")
            w1t = pool.tile([64, 3 * 64], BF16, tag="w1")
            # x2 store: [1 lead margin][R1 blocks of BLK][1 tail margin]
            x2t = pool.tile([128, X2N + 92], BF16, tag="x2")
            w2at = pool.tile([128, 27 * 128], BF16, tag="w2a")
            w2bt = pool.tile([128, 27 * 128], BF16, tag="w2b")
            packft = pool.tile([128, 6 + N5], F32, tag="packf")
            packbt = pool.tile([128, 1024], BF16, tag="packb")
            x3t = pool.tile([128, N3], BF16, tag="x3")
            x4t = pool.tile([128, 2 * N3], BF16, tag="x4")
            x4bt = pool.tile([128, 2 * N3], BF16, tag="x4b")
            x5t = pool.tile([128, N5], F32, tag="x5")
            prodt = pool.tile([128, N5], F32, tag="prod")
            fct = pool.tile([128, 1], F32, tag="fc")
            wut = pool.tile([128, 512], BF16, tag="wu")

            loop_cm = (tc.For_i(0, loop_n) if loop_n > 1
                       else contextlib.nullcontext())
            with loop_cm:
                _build_body(nc, tc, pp, locals(), _stage)

    nc.compile()
    return nc


def _build_body(nc, tc, pp, lv, stage="full"):
    Relu = mybir.ActivationFunctionType.Relu
    (x1t, w1t, x2t, w2at, w2bt, packft, packbt, x3t, x4t, x4bt,
     x5t, prodt, fct, wut) = (
        lv["x1t"], lv["w1t"], lv["x2t"], lv["w2at"], lv["w2bt"],
        lv["packft"], lv["packbt"], lv["x3t"], lv["x4t"], lv["x4bt"],
        lv["x5t"], lv["prodt"], lv["fct"], lv["wut"])
    (x1_d, w1_d, w2a_d, w2b_d, packf_d, packb_d, out_d) = (
        lv["x1_d"], lv["w1_d"], lv["w2a_d"], lv["w2b_d"], lv["packf_d"],
        lv["packb_d"], lv["out_d"])
    if True:
            # PE warm-up burst: the PE clock-gate (HAM) starts at 1.2 GHz and
            # only reaches 2.4 GHz after ~3.4us of sustained activity. The
            # head DMA phase would leave the PE idle anyway, so stream dummy
            # matmuls on a scratch tile to trip the gate before conv1.
            nc.vector.memset(wut[:, :], 0)
            psw = pp.tile([128, 512], F32, tag="ps")
            for wi in range(10):
                nc.tensor.matmul(psw[:, :], wut[:, 0:128], wut[:, :],
                                 start=True, stop=True)
            # SP HWDGE ring: w1 first (conv1's first dependency, tiny), then
            # x1 chunked so early conv1 rows can start before the whole slab
            # lands (Tile subtile deps); first chunk covers rows 0-2, which
            # unblock conv2 row 0.
            nc.sync.dma_start(w1t[:, :], w1_d[:, :])
            for (rlo, rhi) in ((0, 3), (3, 5), (5, 7)):
                nc.sync.dma_start(x1t[:, rlo * 1620:rhi * 1620],
                                  x1_d[:, rlo * 1620:rhi * 1620])
            # ACT HWDGE ring (parallel with SP): w2 in natural tap order
            # (conv2 consumes taps 4,0,1,2,...), interleaved a/b in thirds
            # so every slice lands ~3-7us before conv2 consumes it, then the
            # packed small tensors.
            for (tl, th) in ((0, 9), (9, 18), (18, 27)):
                nc.scalar.dma_start(w2at[:, tl * 128:th * 128],
                                    w2a_d[:, tl * 128:th * 128])
                nc.scalar.dma_start(w2bt[:, tl * 128:th * 128],
                                    w2b_d[:, tl * 128:th * 128])
            nc.scalar.dma_start(packft[:, :], packf_d[:, :])
            nc.scalar.dma_start(packbt[:, :], packb_d[:, :])

            # zero x2: lead margin, tail margin, d4 pad columns. The valid
            # [blk, d2, d3, 0:9] region is fully written by conv1 (dummy edge
            # rows produce exact zeros via the all-ones bias channel).
            nc.vector.memset(x2t[:, 0:1], 0)
            nc.vector.memset(x2t[:, 1 + X2N:X2N + 92], 0)
            x2pad = x2t[:, 1:1 + X2N].rearrange(
                "p (r c) -> p r c", r=R1 * 81, c=D4P)[:, :, 9:10]
            nc.vector.memset(x2pad, 0)

            if stage == "dma":
                nc.vector.memset(fct[:, :], 0)
                nc.sync.dma_start(out_d[:, :], fct[:, :])
                return

            # ---------------- conv1 ----------------
            # slab view: [r(R1), o2(9), o3(9), d4(20)]
            s1v = x1t.rearrange("p (r a b c) -> p r a b c", r=R1, a=9, b=9, c=20)
            # x2 logical view (alloc offset 1): [blk(R1), d2(9), d3(9), d4(D4P)]
            x2v = x2t[:, 1:1 + X2N].rearrange(
                "p (r a b c) -> p r a b c", r=R1, a=9, b=9, c=D4P)
            for r in range(R1):
                for gi, (o2s, c2g) in enumerate(((0, 5), (5, 4))):
                    n = c2g * 81
                    ps1 = pp.tile([128, 512], F32, tag="ps")
                    ps1v = ps1[0:64, :n].rearrange("p (a b c) -> p a b c",
                                                   a=c2g, b=9, c=9)
                    for j4 in range(3):
                        rhs = s1v[:, r, o2s:o2s + c2g, :, j4:j4 + 17:2]
                        nc.tensor.matmul(
                            ps1v[:, :, :, :],
                            w1t[:, j4 * 64:(j4 + 1) * 64],
                            rhs,
                            start=(j4 == 0), stop=(j4 == 2))
                    # (NOTE: splitting these drains ScalarE/VectorE measured
                    # +12us — DVE with a strided PSUM source is much slower
                    # than ACT here; keep all conv1 drains on ScalarE.)
                    nc.scalar.activation(
                        x2v[0:64, r, o2s:o2s + c2g, :, 0:9],
                        ps1v[:, :, :, :],
                        Relu)
                # shifted replica for conv2 j4-fusion, chunked so conv2 can
                # start early while conv1 still runs: x2t[64+p, a] =
                # x2t[p, a+1]. Blocks 0-1 unblock conv2's first 9 taps
                # (row-set (0,1), j1=0); block 2 is needed ~7us later (j1=1
                # taps); the rest follows. Chunks are self-contained; the
                # col a=X2N boundary is pad (zero).
                if r in (1, 2, R1 - 1):
                    clo = {1: 0, 2: 2 * BLK, R1 - 1: 3 * BLK}[r]
                    chi = {1: 2 * BLK, 2: 3 * BLK, R1 - 1: R1 * BLK}[r]
                    nc.sync.dma_start(x2t[64:128, clo:chi],
                                      x2t[0:64, 1 + clo:1 + chi])

            if stage == "c1":
                nc.vector.memset(fct[:, :], 0)
                nc.sync.dma_start(out_d[:, :], fct[:, :])
                return

            # ---------------- conv2 ----------------
            # taps ordered interior-first so the first matmul of each PSUM
            # group covers the full region (has_written correctness).
            # (0,1,1) is interior in (j2,j3) — full region — and its t27=4
            # sits in the first w2 DMA chunk, so the natural-order stream
            # pipelines behind the weight loads.
            taps = sorted(itertools.product(range(3), repeat=3),
                          key=lambda t: (t != (0, 1, 1)))
            x3v = x3t.rearrange("p (r a b c) -> p r a b c", r=R2, a=9, b=9, c=9)
            G2 = ((0, 5), (5, 4))

            def c2geom(j1, j2, j3, r, o2s, c2g):
                lo2 = max(o2s, 1 - j2)
                hi2 = min(o2s + c2g, 10 - j2)
                lo3 = max(0, 1 - j3)
                hi3 = min(9, 10 - j3)
                c2, c3 = hi2 - lo2, hi3 - lo3
                # alloc base for (o2=lo2, o3=lo3, o4=0), j4=0 on the base
                # partitions (the +1 alloc offset and the -1 d4 pad shift
                # cancel):
                base0 = ((r + j1) * BLK + (lo2 + j2 - 1) * 90
                         + (lo3 + j3 - 1) * D4P)
                return lo2, hi2, lo3, hi3, c2, c3, base0

            # Tap-major over row-pairs: one weight load serves 4 matmuls
            # (2 rows x 2 column groups), cutting LDWEIGHTS count 540->162
            # and per-matmul DMA-semaphore waits 4x.
            for rset in ((0, 1), (2, 3), (4,)):
                pss = {}
                for r in rset:
                    for gi, (o2s, c2g) in enumerate(G2):
                        ps2 = pp.tile([128, 512], F32, tag="ps")
                        pss[(r, gi)] = ps2[:, :c2g * 81].rearrange(
                            "p (a b c) -> p a b c", a=c2g, b=9, c=9)
                for ti, (j1, j2, j3) in enumerate(taps):
                    t27 = j1 * 9 + j2 * 3 + j3
                    wa = w2at[:, t27 * 128:(t27 + 1) * 128]
                    wb = w2bt[:, t27 * 128:(t27 + 1) * 128]
                    # ti==0: pair matmul first (full region carries start /
                    # has_written); ti==26: pair matmul last (carries stop).
                    # j4=2 matmul: K=128 at base+2 (w2b rows 64..127 are
                    # zero so the shifted-replica partitions contribute 0),
                    # o4 clipped to [0,8) since o4=8 only reads the d4 pad.
                    for wsel in ((0, 1) if ti == 0 else (1, 0)):
                        for r in rset:
                            for gi, (o2s, c2g) in enumerate(G2):
                                (lo2, hi2, lo3, hi3, c2, c3,
                                 base0) = c2geom(j1, j2, j3, r, o2s, c2g)
                                if wsel == 0:
                                    rhs = x2t[:, base0:base0 +
                                              c2 * 90].rearrange(
                                        "p (a b c) -> p a b c",
                                        a=c2, b=9, c=D4P)[:, :, 0:c3, 0:9]
                                    nc.tensor.matmul(
                                        pss[(r, gi)][:, lo2 - o2s:hi2 - o2s,
                                                     lo3:hi3, :],
                                        wa, rhs, start=(ti == 0),
                                        stop=(ti == 26))
                                else:
                                    rhs = x2t[:, base0 + 2:base0 + 2 +
                                              c2 * 90].rearrange(
                                        "p (a b c) -> p a b c",
                                        a=c2, b=9, c=D4P)[:, :, 0:c3, 0:8]
                                    nc.tensor.matmul(
                                        pss[(r, gi)][:, lo2 - o2s:hi2 - o2s,
                                                     lo3:hi3, 0:8],
                                        wb, rhs, start=False, stop=False)
                for r in rset:
                    for gi, (o2s, c2g) in enumerate(G2):
                        nc.scalar.activation(
                            x3v[:, r, o2s:o2s + c2g, :, :],
                            pss[(r, gi)][:, :, :, :],
                            Relu, bias=packft[:, 0:1])

            if stage == "c2":
                nc.vector.memset(fct[:, :], 0)
                nc.sync.dma_start(out_d[:, :], fct[:, :])
                return

            # ---------------- conv3 (1x1, 64c->128c) ----------------
            chunks = []
            pos = 0
            while pos < N3:
                sz = min(512, N3 - pos)
                chunks.append((pos, sz))
                pos += sz
            # chunk-major (mh inner) so conv4's chunk c — which needs BOTH mh
            # halves of x4t chunk c — can start right after conv3 chunk c.
            # Drains alternate ScalarE/VectorE to split the PSUM->SBUF
            # relu+bias work across both engines.
            Add, Max = mybir.AluOpType.add, mybir.AluOpType.max
            for ci, (pos, sz) in enumerate(chunks):
                for mh in range(2):
                    ps3 = pp.tile([128, 512], F32, tag="ps")
                    nc.tensor.matmul(
                        ps3[:, :sz],
                        packbt[:, mh * 128:(mh + 1) * 128],
                        x3t[:, pos:pos + sz],
                        start=True, stop=True)
                    dst = x4t[:, mh * N3 + pos:mh * N3 + pos + sz]
                    if mh == 0:
                        nc.scalar.activation(dst, ps3[:, :sz],
                                             Relu, bias=packft[:, 1 + mh:2 + mh])
                    else:
                        nc.vector.tensor_scalar(dst, ps3[:, :sz],
                                                packft[:, 1 + mh:2 + mh], 0.0,
                                                Add, Max)

            if stage == "c3":
                nc.vector.memset(fct[:, :], 0)
                nc.sync.dma_start(out_d[:, :], fct[:, :])
                return

            # ---------------- conv4 (1x1, 128c->128c) ----------------
            for ci, (pos, sz) in enumerate(chunks):
                for mh in range(2):
                    ps4 = pp.tile([128, 512], F32, tag="ps")
                    nc.tensor.matmul(
                        ps4[:, :sz],
                        packbt[:, 256 + (mh * 2) * 128:256 + (mh * 2 + 1) * 128],
                        x4t[:, pos:pos + sz],
                        start=True, stop=False)
                    nc.tensor.matmul(
                        ps4[:, :sz],
                        packbt[:, 256 + (mh * 2 + 1) * 128:256 + (mh * 2 + 2) * 128],
                        x4t[:, N3 + pos:N3 + pos + sz],
                        start=False, stop=True)
                    dst = x4bt[:, mh * N3 + pos:mh * N3 + pos + sz]
                    if mh == 0:
                        nc.scalar.activation(dst, ps4[:, :sz],
                                             Relu, bias=packft[:, 3 + mh:4 + mh])
                    else:
                        nc.vector.tensor_scalar(dst, ps4[:, :sz],
                                                packft[:, 3 + mh:4 + mh], 0.0,
                                                Add, Max)

            if stage == "c4":
                nc.vector.memset(fct[:, :], 0)
                nc.sync.dma_start(out_d[:, :], fct[:, :])
                return

            # ---------------- conv5 (1x1, s=2, 128c->64c) ----------------
            # x4b view: [mb(2), r(R2), o2(9), o3(9), o4(9)]
            x4bv = x4bt.rearrange("p (m r a b c) -> p m r a b c",
                                  m=2, r=R2, a=9, b=9, c=9)
            for rr in range(R5):
                ps5 = pp.tile([128, 512], F32, tag="ps")
                for mb in range(2):
                    rhs = x4bv[:, mb, 2 * rr, 0:9:2, 0:9:2, 0:9:2]
                    nc.tensor.matmul(
                        ps5[:, :125],
                        packbt[:, 768 + mb * 128:768 + (mb + 1) * 128],
                        rhs,
                        start=(mb == 0), stop=(mb == 1))
                nc.scalar.activation(
                    x5t[:, rr * 125:(rr + 1) * 125],
                    ps5[:, :125],
                    Relu, bias=packft[:, 5:6])

            if stage == "c5":
                nc.vector.memset(fct[:, :], 0)
                nc.sync.dma_start(out_d[:, :], fct[:, :])
                return

            # ---------------- FC partials ----------------
            # (tensor_tensor_reduce would fuse these but faults in this
            # execution path)
            nc.vector.tensor_mul(prodt[:, :], x5t[:, :], packft[:, 6:6 + N5])
            nc.vector.reduce_sum(fct[:, :], prodt[:, :],
                                 axis=mybir.AxisListType.X)

            nc.sync.dma_start(out_d[:, :], fct[:, :], single_packet=True)


# ---------------- host-side data prep ----------------

def _prep_weights(inputs):
    f32 = np.float32
    w1r = np.asarray(inputs["w1r"], f32)[:, 0]   # [32, 3,3,3,3]
    w1i = np.asarray(inputs["w1i"], f32)[:, 0]
    # [t27, j4, co]
    w1r_t = w1r.transpose(1, 2, 3, 4, 0).reshape(27, 3, 32)
    w1i_t = w1i.transpose(1, 2, 3, 4, 0).reshape(27, 3, 32)
    W1 = np.zeros((64, 3 * 64), f32)
    for j4 in range(3):
        W1[0:27, j4 * 64:j4 * 64 + 32] = w1r_t[:, j4]
        W1[0:27, j4 * 64 + 32:j4 * 64 + 64] = w1i_t[:, j4]
        W1[27:54, j4 * 64:j4 * 64 + 32] = -w1i_t[:, j4]
        W1[27:54, j4 * 64 + 32:j4 * 64 + 64] = w1r_t[:, j4]
    W1[54, 0:32] = np.asarray(inputs["b1r"], f32)
    W1[54, 32:64] = np.asarray(inputs["b1i"], f32)

    w2r = np.asarray(inputs["w2r"], f32)   # [64, 32, 3,3,3,3]
    w2i = np.asarray(inputs["w2i"], f32)
    # [t27, j4, ci, co]
    w2r_t = w2r.transpose(2, 3, 4, 5, 1, 0).reshape(27, 3, 32, 64)
    w2i_t = w2i.transpose(2, 3, 4, 5, 1, 0).reshape(27, 3, 32, 64)
    W2a = np.zeros((128, 27 * 128), f32)
    W2b = np.zeros((128, 27 * 128), f32)  # rows 64..127 stay 0 (K=128 pad)
    for t in range(27):
        for jj, r0 in ((0, 0), (1, 64)):
            W2a[r0 + 0:r0 + 32, t * 128:t * 128 + 64] = w2r_t[t, jj]
            W2a[r0 + 0:r0 + 32, t * 128 + 64:(t + 1) * 128] = w2i_t[t, jj]
            W2a[r0 + 32:r0 + 64, t * 128:t * 128 + 64] = -w2i_t[t, jj]
            W2a[r0 + 32:r0 + 64, t * 128 + 64:(t + 1) * 128] = w2r_t[t, jj]
        W2b[0:32, t * 128:t * 128 + 64] = w2r_t[t, 2]
        W2b[0:32, t * 128 + 64:(t + 1) * 128] = w2i_t[t, 2]
        W2b[32:64, t * 128:t * 128 + 64] = -w2i_t[t, 2]
        W2b[32:64, t * 128 + 64:(t + 1) * 128] = w2r_t[t, 2]
    B2 = np.concatenate([np.asarray(inputs["b2r"], f32),
                         np.asarray(inputs["b2i"], f32)])[:, None]

    w3r = np.asarray(inputs["w3r"], f32).reshape(128, 64)
    w3i = np.asarray(inputs["w3i"], f32).reshape(128, 64)
    W3 = np.zeros((128, 2 * 128), f32)
    W3[0:64, 0:128] = w3r.T
    W3[64:128, 0:128] = -w3i.T
    W3[0:64, 128:256] = w3i.T
    W3[64:128, 128:256] = w3r.T
    B3 = np.stack([np.asarray(inputs["b3r"], f32),
                   np.asarray(inputs["b3i"], f32)], axis=1)

    w4r = np.asarray(inputs["w4r"], f32).reshape(128, 128)
    w4i = np.asarray(inputs["w4i"], f32).reshape(128, 128)
    W4 = np.zeros((128, 4 * 128), f32)
    W4[:, 0:128] = w4r.T
    W4[:, 128:256] = -w4i.T
    W4[:, 256:384] = w4i.T
    W4[:, 384:512] = w4r.T
    B4 = np.stack([np.asarray(inputs["b4r"], f32),
                   np.asarray(inputs["b4i"], f32)], axis=1)

    w5r = np.asarray(inputs["w5r"], f32).reshape(64, 128)
    w5i = np.asarray(inputs["w5i"], f32).reshape(64, 128)
    W5 = np.zeros((128, 2 * 128), f32)
    W5[:, 0:64] = w5r.T
    W5[:, 64:128] = w5i.T
    W5[:, 128:192] = -w5i.T
    W5[:, 192:256] = w5r.T
    B5 = np.concatenate([np.asarray(inputs["b5r"], f32),
                         np.asarray(inputs["b5i"], f32)])[:, None]

    # packf cols: [0]=b2, [1:3]=b3, [3:5]=b4, [5:6]=b5 (fcw appended
    # per-core by _mk_packf); packb cols: [0:256]=w3, [256:768]=w4,
    # [768:1024]=w5
    packf0 = np.concatenate([B2, B3, B4, B5], axis=1).astype(f32)
    packb = np.concatenate([W3, W4, W5], axis=1).astype(BF)
    return {
        "w1": W1.astype(BF), "w2a": W2a.astype(BF), "w2b": W2b.astype(BF),
        "packf0": packf0, "packb": packb,
    }


def _mk_packf(packf0, fcw, h):
    return np.concatenate([packf0, _prep_fcw(fcw, h)], axis=1)


def _prep_x1(xr_b, xi_b, h):
    """Conv1 input slab for one (batch, half): [64, R1, 9, 9, 20] bf16."""
    S = np.zeros((64, R1, 9, 9, 20), np.float32)
    glo = max(0, 4 * h - 1)
    ghi = min(8, 4 * h + 5)
    rlo = glo - (4 * h - 1)
    rhi = ghi - (4 * h - 1) + 1
    for t, (j1, j2, j3) in enumerate(itertools.product(range(3), repeat=3)):
        subr = xr_b[j1:j1 + 17:2, j2:j2 + 17:2, j3:j3 + 17:2, :]
        subi = xi_b[j1:j1 + 17:2, j2:j2 + 17:2, j3:j3 + 17:2, :]
        S[t, rlo:rhi] = subr[glo:ghi + 1]
        S[27 + t, rlo:rhi] = subi[glo:ghi + 1]
    S[54, rlo:rhi] = 1.0
    return S.reshape(64, S1N).astype(BF)


def _prep_fcw(fcw, h):
    out = np.zeros((128, N5), np.float32)
    f = np.asarray(fcw, np.float32).reshape(-1)
    for rr in range(R5):
        g5 = rr + 2 * h
        if h == 1 and rr == 0:
            continue  # overlap row masked on half 1
        out[:, rr * 125:(rr + 1) * 125] = f[g5 * 125:(g5 + 1) * 125][None, :]
    return out


def kernel(**inputs):
    if "nc" not in _CACHE:
        _CACHE["nc"] = _build_nc()
    nc = _CACHE["nc"]

    wmaps = _prep_weights(inputs)
    xr = np.asarray(inputs["xr"], np.float32)
    xi = np.asarray(inputs["xi"], np.float32)
    fcw = inputs["fcw"]

    in_maps = []
    for core in range(8):
        b, h = core // 2, core % 2
        m = {"w1": wmaps["w1"], "w2a": wmaps["w2a"], "w2b": wmaps["w2b"],
             "packb": wmaps["packb"]}
        m["x1"] = _prep_x1(xr[b, 0], xi[b, 0], h)
        m["packf"] = _mk_packf(wmaps["packf0"], fcw, h)
        in_maps.append(m)

    res = run_bass_kernel_spmd(nc, in_maps, core_ids=list(range(8)))

    fcb = np.asarray(inputs["fcb"], np.float32)
    yr = np.zeros((NB, 64, 1), np.float32)
    yi = np.zeros((NB, 64, 1), np.float32)
    for b in range(NB):
        p0 = res.results[2 * b]["out"]
        p1 = res.results[2 * b + 1]["out"]
        s = p0 + p1
        yr[b] = s[0:64] + fcb[0]
        yi[b] = s[64:128]
    return np.stack([yr, yi]).astype(np.float32)

